# revision 11
# baseline (speedup 1.0000x reference)
"""Trainium2 Bass kernel: conv2d(3->16, 3x3, valid) + bias + exact GELU + global mean pool.

Input  x: [128, 3, 256, 256] f32  ->  output [128, 16] f32.

Strategy (pure data parallel over 8 NeuronCores, 16 images/core):
  * Host packs each image into a "quad" layout so the 3x3 conv becomes 6
    PSUM-accumulated matmuls per output-row block:
      partitions p = c*40 + q*10 + ri   (c: in-channel, q: column mod 4, ri: row in block)
      free dims  = (blk: 32 row-blocks, u: 64 column-quads + 1 zero pad)
    Row 120 is an indicator (1.0 at u=63) which, through a -1e30 stationary
    weight, drives the two phantom outputs (j=254/255) to -inf so GELU maps
    them to exactly 0. Row 121 is 1.0 only for the tail row-block (31), and
    kills its duplicated output rows (ro<2) the same way.
  * Device: per image: one SWDGE casting DMA (DRAM f32 -> SBUF bf16), then
    per group of 8 row-blocks: 6 matmuls (N=512, one PSUM bank each) into a
    4-bank PSUM tile (double buffered), then ScalarE Gelu (per-partition bias
    fused) over the whole tile. Pooling partials alternate between ScalarE's
    accum_out and a VectorE tensor_reduce to balance the two engines.
  * A final selector matmul folds the 1/(254*254) mean and emits [16, 16].
"""

import numpy as np
import ml_dtypes

B, C_IN, H, W = 128, 3, 256, 256
C_OUT, K = 16, 3
HO, WO = H - K + 1, W - K + 1  # 254, 254
N_CORES = 8
IMG_PER_CORE = B // N_CORES  # 16
NBLK = 32          # row blocks per image (31 full + tail)
RPB = 8            # output rows per block
RI = 10            # input rows per block
NQ = 4             # column quads
NU = 64            # u positions per row (W/4)
KDIM = 122         # 120 data + phantom indicator + tail indicator
MDIM = 128         # 16 out-channels x 8 rows
GPB = 8            # blocks per psum group (4-bank tile)
NGRP = NBLK // GPB   # 4 groups per image
BIG_NEG = -1.0e30

# taps per stationary matrix: list of (q, dj) pairs
W_TAPS = [
    [(0, 0), (1, 1), (2, 2)],  # W0 -> qo0, shift 0
    [(1, 0), (2, 1), (3, 2)],  # W1 -> qo1, shift 0
    [(2, 0), (3, 1)],          # W2 -> qo2, shift 0 (start)
    [(0, 2)],                  # W3 -> qo2, shift 1 (stop)
    [(3, 0)],                  # W4 -> qo3, shift 0 (start)
    [(0, 1), (1, 2)],          # W5 -> qo3, shift 1 (stop)
]
# per stationary: (qo region, rhs shift, start, stop)
W_INFO = [
    (0, 0, True, True),
    (1, 0, True, True),
    (2, 0, True, False),
    (2, 1, False, True),
    (3, 0, True, False),
    (3, 1, False, True),
]
PHANTOM_KILLERS = (2, 4)      # W idx carrying -1e30 on row 120 (all columns)
TAIL_KILLERS = (0, 1, 2, 4)   # W idx carrying -1e30 on row 121 (columns ro<2)


def _pack_x_shard(xs: np.ndarray) -> np.ndarray:
    """xs: [IMG, 3, 256, 256] f32 -> [IMG, 122, 32, 65] f32 quad-packed."""
    n_img = xs.shape[0]
    bases = np.array([8 * b for b in range(NBLK - 1)] + [H - RI], dtype=np.int64)
    rows = bases[:, None] + np.arange(RI)[None, :]          # [32, 10]
    tmp = xs[:, :, rows, :]                                  # [IMG, 3, 32, 10, 256]
    tmp = tmp.reshape(n_img, C_IN, NBLK, RI, NU, NQ)         # col = 4u + q
    tmp = tmp.transpose(0, 1, 5, 3, 2, 4)                    # [IMG, c, q, ri, blk, u]
    packed = np.zeros((n_img, KDIM, NBLK, NU + 1), dtype=np.float32)
    packed[:, :120, :, :NU] = tmp.reshape(n_img, 120, NBLK, NU)
    packed[:, 120, :, NU - 1] = 1.0   # phantom indicator (u = 63)
    packed[:, 121, NBLK - 1, :] = 1.0  # tail-block indicator
    return packed


def _build_weights(weight: np.ndarray) -> np.ndarray:
    """weight: [16, 3, 3, 3] f32 (OIHW) -> [6, 122, 128] bf16 stationaries."""
    Wt = np.zeros((6, KDIM, MDIM), dtype=np.float32)
    for idx, taps in enumerate(W_TAPS):
        for (q, dj) in taps:
            for di in range(K):
                for ro in range(RPB):
                    ri = ro + di
                    ks = np.arange(C_IN) * 40 + q * 10 + ri          # [3]
                    ms = np.arange(C_OUT) * RPB + ro                  # [16]
                    Wt[idx, ks[:, None], ms[None, :]] = weight[:, :, di, dj].T
    for idx in PHANTOM_KILLERS:
        Wt[idx, 120, :] = BIG_NEG
    ro_mask = (np.arange(MDIM) % RPB) < 2
    for idx in TAIL_KILLERS:
        Wt[idx, 121, ro_mask] = BIG_NEG
    return Wt.astype(ml_dtypes.bfloat16)


def _build_sel() -> np.ndarray:
    inv = np.float32(1.0 / (HO * WO))
    sel = np.zeros((MDIM, C_OUT), dtype=np.float32)
    for o in range(C_OUT):
        sel[o * RPB:(o + 1) * RPB, o] = inv
    return sel


_PROGRAM_CACHE = {}


def _build_program():
    if "nc" in _PROGRAM_CACHE:
        return _PROGRAM_CACHE["nc"]
    import concourse.bass as bass
    import concourse.mybir as mybir
    import concourse.tile as tile
    from concourse import bacc

    f32 = mybir.dt.float32
    f16 = mybir.dt.float16
    bf16 = mybir.dt.bfloat16

    nc = bacc.Bacc("TRN2", target_bir_lowering=False, debug=False,
                   num_devices=N_CORES)

    xp_dram = nc.dram_tensor("xp", [IMG_PER_CORE, KDIM, NBLK, NU + 1], f32,
                             kind="ExternalInput").ap()
    wt_dram = nc.dram_tensor("wt", [6, KDIM, MDIM], bf16,
                             kind="ExternalInput").ap()
    bias_dram = nc.dram_tensor("bias", [MDIM, 1], f32, kind="ExternalInput").ap()
    sel_dram = nc.dram_tensor("sel", [MDIM, C_OUT], f32,
                              kind="ExternalInput").ap()
    out_dram = nc.dram_tensor("out", [IMG_PER_CORE, C_OUT], f32,
                              kind="ExternalOutput").ap()

    n_groups = IMG_PER_CORE * NGRP  # 128 groups of 4 blocks

    with tile.TileContext(nc) as tc:
        with (
            tc.tile_pool(name="consts", bufs=1) as consts,
            tc.tile_pool(name="work", bufs=2) as work,
            tc.tile_pool(name="psum", bufs=2, space="PSUM") as psum,
        ):
            w_sb = consts.tile([KDIM, 6, MDIM], bf16)
            for i in range(6):
                nc.sync.dma_start(w_sb[:, i, :], wt_dram[i])
            bias_sb = consts.tile([MDIM, 1], f32)
            nc.sync.dma_start(bias_sb[:], bias_dram[:])
            sel_sb = consts.tile([MDIM, C_OUT], f32)
            nc.sync.dma_start(sel_sb[:], sel_dram[:])
            pa = consts.tile([MDIM, IMG_PER_CORE, NGRP], f32)

            gelu = mybir.ActivationFunctionType.Gelu
            for img in range(IMG_PER_CORE):
                d = work.tile([KDIM, NBLK, NU + 1], bf16, tag="d", bufs=4)
                nc.gpsimd.dma_start(d[:], xp_dram[img])  # SWDGE casting DMA f32->bf16
                for gi in range(NGRP):
                    b0 = GPB * gi
                    ps = psum.tile([MDIM, NQ, GPB, NU], f32, tag="ps", bufs=2)
                    for wi in range(6):
                        qo, s, st, sp = W_INFO[wi]
                        nc.tensor.matmul(
                            ps[:, qo],
                            w_sb[:, wi, :],
                            d[:, b0:b0 + GPB, s:s + NU],
                            start=st, stop=sp,
                        )
                    gl = work.tile([MDIM, NQ, GPB, NU], f16, tag="gl", bufs=3)
                    if gi % 2 == 0:
                        nc.scalar.activation(gl[:], ps[:], gelu,
                                             bias=bias_sb[:], scale=1.0,
                                             accum_out=pa[:, img, gi:gi + 1])
                    else:
                        nc.scalar.activation(gl[:], ps[:], gelu,
                                             bias=bias_sb[:], scale=1.0)
                        nc.vector.tensor_reduce(
                            out=pa[:, img, gi:gi + 1], in_=gl[:],
                            axis=mybir.AxisListType.XYZ, op=mybir.AluOpType.add,
                        )

            # final: per-image partial sums -> selector matmul -> output
            pm = consts.tile([MDIM, IMG_PER_CORE], f32)
            for img in range(IMG_PER_CORE):
                nc.vector.tensor_reduce(
                    out=pm[:, img:img + 1], in_=pa[:, img, :],
                    axis=mybir.AxisListType.X, op=mybir.AluOpType.add,
                )
            ops = psum.tile([IMG_PER_CORE, C_OUT], f32, tag="ps", bufs=2)
            nc.tensor.matmul(ops[:], pm[:], sel_sb[:], start=True, stop=True)
            res = consts.tile([IMG_PER_CORE, C_OUT], f32)
            nc.vector.tensor_copy(res[:], ops[:])
            nc.sync.dma_start(out_dram[:], res[:])

    nc.compile()
    _PROGRAM_CACHE["nc"] = nc
    return nc


def _prepare_in_maps(x, weight, bias):
    wt = _build_weights(np.asarray(weight, dtype=np.float32))
    sel = _build_sel()
    bias_col = np.repeat(np.asarray(bias, dtype=np.float32), RPB).reshape(MDIM, 1)
    in_maps = []
    for core in range(N_CORES):
        xs = np.asarray(x[core * IMG_PER_CORE:(core + 1) * IMG_PER_CORE],
                        dtype=np.float32)
        in_maps.append({
            "xp": _pack_x_shard(xs),
            "wt": wt,
            "bias": bias_col,
            "sel": sel,
        })
    return in_maps


def run(x, weight, bias, trace=False, tmpdir=None, **kw):
    from concourse.bass_utils import run_bass_kernel_spmd
    nc = _build_program()
    in_maps = _prepare_in_maps(x, weight, bias)
    r = run_bass_kernel_spmd(nc, in_maps, list(range(N_CORES)), trace=trace,
                             tmpdir=tmpdir, **kw)
    out = np.concatenate([r.results[c]["out"] for c in range(N_CORES)], axis=0)
    return out.astype(np.float32), r


def kernel(x, weight, bias):
    out, _ = run(x, weight, bias, trace=False)
    return out
